# revision 6
# baseline (speedup 1.0000x reference)
"""Trainium2 Bass kernel for nn_AdversarialLoss.

Math (per row r of pred [B, V]):
    out[r] = -(sum_v log(pred[r, v]) - log(pred[r, target[r]])) / V
           = (log(pred[r, target[r]]) - rowsum_log[r]) / V

Strategy (8-way data parallel over rows, 1024 rows/core):
  - Stream pred tiles [128 rows x C cols] HBM->SBUF via HWDGE DMA.
  - One ACT (ScalarE) Ln pass per tile with accum_out giving the row-sums
    directly (no separate reduction pass over the data).
  - The 1024 target entries per core are fetched with a gpsimd indirect
    (gather) DMA, fully overlapped with the streaming pass.
  - Tiny DVE/ACT combine at the end, one small DMA out.
"""

import sys

if "/opt/trn_rl_repo" not in sys.path:
    sys.path.insert(0, "/opt/trn_rl_repo")

import numpy as np

B, V = 8192, 32000
NCORES = 8
R = B // NCORES          # rows per core = 1024
P = 128                  # SBUF partitions
NBLK = R // P            # row blocks per core = 8
C = 8000                 # column chunk (free dim) per tile
NCH = V // C             # chunks per row block = 4

_CACHE = {}


def _build_program():
    import concourse.bass as bass
    import concourse.bacc as bacc
    import concourse.tile as tile
    from concourse import mybir

    nc = bacc.Bacc(
        "TRN2", target_bir_lowering=False, debug=False, num_devices=NCORES
    )
    pred = nc.declare_dram_parameter("pred", [R, V], mybir.dt.float32, isOutput=False)
    tidx = nc.declare_dram_parameter("tidx", [R], mybir.dt.int32, isOutput=False)
    out = nc.declare_dram_parameter("out", [R], mybir.dt.float32, isOutput=True)

    pred_flat = pred.reshape([R * V, 1])

    # chunk plan per row-block: big chunks mid-stream; the LAST block tapers
    # so the final ACT (which can't overlap any DMA) is short.
    full_chunks = [C] * NCH
    taper_chunks = [8000, 8000, 8000, 4000, 2000, 1000, 750, 250]
    assert sum(taper_chunks) == V

    with tile.TileContext(nc) as tc:
        with (
            tc.tile_pool(name="stream", bufs=4) as stream,
            tc.tile_pool(name="small", bufs=1) as small,
            tc.tile_pool(name="parts", bufs=2) as parts,
        ):
            # warm-up Ln on a const tile: forces the ACT table load during
            # the DMA startup window instead of stalling the first real tile
            warm = small.tile([P, 1], mybir.dt.float32)
            nc.vector.memset(warm[:], 1.0)
            nc.scalar.activation(
                out=warm[:], in_=warm[:], func=mybir.ActivationFunctionType.Ln
            )

            # --- target-entry gather (overlaps with the streaming pass) ---
            # idx load + indirect gathers all live on the gpsimd SWDGE queue,
            # keeping the HWDGE stream queue free.
            idx_sb = small.tile([P, NBLK], mybir.dt.int32)
            nc.gpsimd.dma_start(
                out=idx_sb[:], in_=tidx[:].rearrange("(b p) -> p b", p=P)
            )
            gath = small.tile([P, NBLK], mybir.dt.float32)
            for b in range(NBLK):
                nc.gpsimd.indirect_dma_start(
                    out=gath[:, b : b + 1],
                    out_offset=None,
                    in_=pred_flat[:],
                    in_offset=bass.IndirectOffsetOnAxis(
                        ap=idx_sb[:, b : b + 1], axis=0
                    ),
                )
                # per-column Ln: one sync-wait per instruction (a single Ln
                # over all 8 columns would need 8 DMA sem waits, over the HW
                # sync-wait limit)
                nc.scalar.activation(
                    out=gath[:, b : b + 1],
                    in_=gath[:, b : b + 1],
                    func=mybir.ActivationFunctionType.Ln,
                )

            # --- streaming log + row-sum pass, combine folded in per block ---
            res = small.tile([P, NBLK], mybir.dt.float32)
            for b in range(NBLK):
                chunks = taper_chunks if b == NBLK - 1 else full_chunks
                partial = parts.tile(
                    [P, len(taper_chunks)], mybir.dt.float32, tag="partial"
                )
                col = 0
                for j, ch in enumerate(chunks):
                    t = stream.tile([P, C], mybir.dt.float32, tag="t")
                    nc.sync.dma_start(
                        out=t[:, :ch],
                        in_=pred[b * P : (b + 1) * P, col : col + ch],
                    )
                    nc.scalar.activation(
                        out=t[:, :ch],
                        in_=t[:, :ch],
                        func=mybir.ActivationFunctionType.Ln,
                        accum_out=partial[:, j : j + 1],
                    )
                    col += ch
                bsum = parts.tile([P, 1], mybir.dt.float32, tag="bsum")
                nc.vector.reduce_sum(
                    out=bsum[:],
                    in_=partial[:, : len(chunks)],
                    axis=mybir.AxisListType.X,
                )
                # res[:, b] = (log(gathered) - rowsum) / V
                nc.vector.tensor_scalar(
                    out=res[:, b : b + 1],
                    in0=gath[:, b : b + 1],
                    scalar1=bsum[:],
                    scalar2=1.0 / V,
                    op0=mybir.AluOpType.subtract,
                    op1=mybir.AluOpType.mult,
                )
            nc.sync.dma_start(
                out=out[:].rearrange("(b p) -> p b", p=P), in_=res[:]
            )

    nc.compile()
    return nc


def _run(pred, target, trace=False, **kwargs):
    from concourse.bass_utils import run_bass_kernel_spmd

    if "nc" not in _CACHE:
        _CACHE["nc"] = _build_program()
    nc = _CACHE["nc"]

    pred = np.ascontiguousarray(np.asarray(pred, dtype=np.float32))
    tgt = np.asarray(target).astype(np.int64).reshape(-1)
    assert pred.shape == (B, V) and tgt.shape == (B,)

    base = np.arange(R, dtype=np.int64) * V
    in_maps = []
    for c in range(NCORES):
        sl = slice(c * R, (c + 1) * R)
        tidx = (base + tgt[sl]).astype(np.int32)
        in_maps.append({"pred": pred[sl], "tidx": tidx})

    res = run_bass_kernel_spmd(
        nc, in_maps, core_ids=list(range(NCORES)), trace=trace, **kwargs
    )
    out = np.concatenate([np.asarray(r["out"]).reshape(-1) for r in res.results])
    return out, res


def kernel(pred, target):
    return _run(pred, target)[0]


# revision 8
# speedup vs baseline: 1.0351x; 1.0351x over previous
"""Trainium2 Bass kernel for nn_AdversarialLoss.

Math (per row r of pred [B, V]):
    out[r] = -(sum_v log(pred[r, v]) - log(pred[r, target[r]])) / V
           = (log(pred[r, target[r]]) - rowsum_log[r]) / V

Strategy (8-way data parallel over rows, 1024 rows/core):
  - Stream pred tiles [128 rows x C cols] HBM->SBUF via HWDGE DMA.
  - One ACT (ScalarE) Ln pass per tile with accum_out giving the row-sums
    directly (no separate reduction pass over the data).
  - The 1024 target entries per core are fetched with a gpsimd indirect
    (gather) DMA, fully overlapped with the streaming pass.
  - Tiny DVE/ACT combine at the end, one small DMA out.
"""

import sys

if "/opt/trn_rl_repo" not in sys.path:
    sys.path.insert(0, "/opt/trn_rl_repo")

import numpy as np

B, V = 8192, 32000
NCORES = 8
R = B // NCORES          # rows per core = 1024
P = 128                  # SBUF partitions
NBLK = R // P            # row blocks per core = 8
C = 8000                 # column chunk (free dim) per tile
NCH = V // C             # chunks per row block = 4

_CACHE = {}


def _build_program():
    import concourse.bass as bass
    import concourse.bacc as bacc
    import concourse.tile as tile
    from concourse import mybir

    nc = bacc.Bacc(
        "TRN2", target_bir_lowering=False, debug=False, num_devices=NCORES
    )
    pred = nc.declare_dram_parameter("pred", [R, V], mybir.dt.float32, isOutput=False)
    tidx = nc.declare_dram_parameter("tidx", [R], mybir.dt.int32, isOutput=False)
    out = nc.declare_dram_parameter("out", [R], mybir.dt.float32, isOutput=True)

    pred_flat = pred.reshape([R * V, 1])

    # chunk plan per row-block: big chunks mid-stream; the LAST block tapers
    # so the final ACT (which can't overlap any DMA) is short.
    full_chunks = [C] * NCH
    taper_chunks = [8000, 8000, 8000, 4000, 2000, 1000, 750, 250]
    assert sum(taper_chunks) == V

    from concourse.masks import make_identity

    with tile.TileContext(nc) as tc:
        with (
            tc.tile_pool(name="stream", bufs=4) as stream,
            tc.tile_pool(name="small", bufs=1) as small,
            tc.tile_pool(name="parts", bufs=2) as parts,
            tc.tile_pool(name="psum", bufs=1, space="PSUM") as psum,
        ):
            # identity for the final PE transpose of the result tile
            identity = small.tile([P, P], mybir.dt.float32)
            make_identity(nc, identity[:])
            # warm-up Ln on a const tile: forces the ACT table load during
            # the DMA startup window instead of stalling the first real tile
            warm = small.tile([P, 1], mybir.dt.float32)
            nc.vector.memset(warm[:], 1.0)
            nc.scalar.activation(
                out=warm[:], in_=warm[:], func=mybir.ActivationFunctionType.Ln
            )

            # --- target-entry gather (overlaps with the streaming pass) ---
            # idx load + indirect gathers all live on the gpsimd SWDGE queue,
            # keeping the HWDGE stream queue free.
            idx_sb = small.tile([P, NBLK], mybir.dt.int32)
            nc.gpsimd.dma_start(
                out=idx_sb[:], in_=tidx[:].rearrange("(b p) -> p b", p=P)
            )
            gath = small.tile([P, NBLK], mybir.dt.float32)
            for b in range(NBLK):
                nc.gpsimd.indirect_dma_start(
                    out=gath[:, b : b + 1],
                    out_offset=None,
                    in_=pred_flat[:],
                    in_offset=bass.IndirectOffsetOnAxis(
                        ap=idx_sb[:, b : b + 1], axis=0
                    ),
                )
                # per-column Ln: one sync-wait per instruction (a single Ln
                # over all 8 columns would need 8 DMA sem waits, over the HW
                # sync-wait limit)
                nc.scalar.activation(
                    out=gath[:, b : b + 1],
                    in_=gath[:, b : b + 1],
                    func=mybir.ActivationFunctionType.Ln,
                )

            # --- streaming log + row-sum pass, combine folded in per block ---
            res = small.tile([P, NBLK], mybir.dt.float32)
            for b in range(NBLK):
                chunks = taper_chunks if b == NBLK - 1 else full_chunks
                partial = parts.tile(
                    [P, len(taper_chunks)], mybir.dt.float32, tag="partial"
                )
                col = 0
                for j, ch in enumerate(chunks):
                    t = stream.tile([P, C], mybir.dt.float32, tag="t")
                    nc.sync.dma_start(
                        out=t[:, :ch],
                        in_=pred[b * P : (b + 1) * P, col : col + ch],
                    )
                    nc.scalar.activation(
                        out=t[:, :ch],
                        in_=t[:, :ch],
                        func=mybir.ActivationFunctionType.Ln,
                        accum_out=partial[:, j : j + 1],
                    )
                    col += ch
                bsum = parts.tile([P, 1], mybir.dt.float32, tag="bsum")
                nc.vector.reduce_sum(
                    out=bsum[:],
                    in_=partial[:, : len(chunks)],
                    axis=mybir.AxisListType.X,
                )
                # res[:, b] = (log(gathered) - rowsum) / V
                nc.vector.tensor_scalar(
                    out=res[:, b : b + 1],
                    in0=gath[:, b : b + 1],
                    scalar1=bsum[:],
                    scalar2=1.0 / V,
                    op0=mybir.AluOpType.subtract,
                    op1=mybir.AluOpType.mult,
                )
            # transpose res [128, 8] -> [8, 128] so the output DMA writes
            # 8 contiguous 512B runs instead of 1024 scattered 4B RMW writes
            # (the scattered form cost ~7.5us of write-receipt latency in the
            # kernel-tail drain)
            resT_psum = psum.tile([NBLK, P], mybir.dt.float32)
            nc.tensor.transpose(
                out=resT_psum[:], in_=res[:], identity=identity[:]
            )
            resT = small.tile([NBLK, P], mybir.dt.float32)
            nc.vector.tensor_copy(out=resT[:], in_=resT_psum[:])
            nc.sync.dma_start(
                out=out[:].rearrange("(b p) -> b p", p=P), in_=resT[:]
            )

    nc.compile()
    return nc


def _run(pred, target, trace=False, **kwargs):
    from concourse.bass_utils import run_bass_kernel_spmd

    if "nc" not in _CACHE:
        _CACHE["nc"] = _build_program()
    nc = _CACHE["nc"]

    pred = np.ascontiguousarray(np.asarray(pred, dtype=np.float32))
    tgt = np.asarray(target).astype(np.int64).reshape(-1)
    assert pred.shape == (B, V) and tgt.shape == (B,)

    base = np.arange(R, dtype=np.int64) * V
    in_maps = []
    for c in range(NCORES):
        sl = slice(c * R, (c + 1) * R)
        tidx = (base + tgt[sl]).astype(np.int32)
        in_maps.append({"pred": pred[sl], "tidx": tidx})

    res = run_bass_kernel_spmd(
        nc, in_maps, core_ids=list(range(NCORES)), trace=trace, **kwargs
    )
    out = np.concatenate([np.asarray(r["out"]).reshape(-1) for r in res.results])
    return out, res


def kernel(pred, target):
    return _run(pred, target)[0]


# revision 9
# speedup vs baseline: 1.0386x; 1.0034x over previous
"""Trainium2 Bass kernel for nn_AdversarialLoss.

Math (per row r of pred [B, V]):
    out[r] = -(sum_v log(pred[r, v]) - log(pred[r, target[r]])) / V
           = (log(pred[r, target[r]]) - rowsum_log[r]) / V

Strategy (8-way data parallel over rows, 1024 rows/core):
  - Stream pred tiles [128 rows x C cols] HBM->SBUF via HWDGE DMA.
  - One ACT (ScalarE) Ln pass per tile with accum_out giving the row-sums
    directly (no separate reduction pass over the data).
  - The 1024 target entries per core are fetched with a gpsimd indirect
    (gather) DMA, fully overlapped with the streaming pass.
  - Tiny DVE/ACT combine at the end, one small DMA out.
"""

import sys

if "/opt/trn_rl_repo" not in sys.path:
    sys.path.insert(0, "/opt/trn_rl_repo")

import numpy as np

B, V = 8192, 32000
NCORES = 8
R = B // NCORES          # rows per core = 1024
P = 128                  # SBUF partitions
NBLK = R // P            # row blocks per core = 8
C = 8000                 # column chunk (free dim) per tile
NCH = V // C             # chunks per row block = 4

_CACHE = {}


def _build_program():
    import concourse.bass as bass
    import concourse.bacc as bacc
    import concourse.tile as tile
    from concourse import mybir

    nc = bacc.Bacc(
        "TRN2", target_bir_lowering=False, debug=False, num_devices=NCORES
    )
    pred = nc.declare_dram_parameter("pred", [R, V], mybir.dt.float32, isOutput=False)
    tidx = nc.declare_dram_parameter("tidx", [R], mybir.dt.int32, isOutput=False)
    out = nc.declare_dram_parameter("out", [R], mybir.dt.float32, isOutput=True)

    pred_flat = pred.reshape([R * V, 1])

    # chunk plan per row-block: big chunks mid-stream; the LAST block tapers
    # so the final ACT (which can't overlap any DMA) is short.
    full_chunks = [C] * NCH
    taper_chunks = [8000, 8000, 8000, 4000, 2000, 1000, 750, 250]
    assert sum(taper_chunks) == V

    from concourse.masks import make_identity

    with tile.TileContext(nc) as tc:
        with (
            tc.tile_pool(name="stream", bufs=5) as stream,
            tc.tile_pool(name="small", bufs=1) as small,
            tc.tile_pool(name="parts", bufs=2) as parts,
            tc.tile_pool(name="psum", bufs=1, space="PSUM") as psum,
        ):
            # identity for the final PE transpose of the result tile
            identity = small.tile([P, P], mybir.dt.float32)
            make_identity(nc, identity[:])
            # warm-up Ln on a const tile: forces the ACT table load during
            # the DMA startup window instead of stalling the first real tile
            warm = small.tile([P, 1], mybir.dt.float32)
            nc.vector.memset(warm[:], 1.0)
            nc.scalar.activation(
                out=warm[:], in_=warm[:], func=mybir.ActivationFunctionType.Ln
            )

            # --- target-entry gather (overlaps with the streaming pass) ---
            # idx load + indirect gathers all live on the gpsimd SWDGE queue,
            # keeping the HWDGE stream queue free.
            idx_sb = small.tile([P, NBLK], mybir.dt.int32)
            nc.gpsimd.dma_start(
                out=idx_sb[:], in_=tidx[:].rearrange("(b p) -> p b", p=P)
            )
            gath = small.tile([P, NBLK], mybir.dt.float32)
            for b in range(NBLK):
                nc.gpsimd.indirect_dma_start(
                    out=gath[:, b : b + 1],
                    out_offset=None,
                    in_=pred_flat[:],
                    in_offset=bass.IndirectOffsetOnAxis(
                        ap=idx_sb[:, b : b + 1], axis=0
                    ),
                )
                # per-column Ln: one sync-wait per instruction (a single Ln
                # over all 8 columns would need 8 DMA sem waits, over the HW
                # sync-wait limit)
                nc.scalar.activation(
                    out=gath[:, b : b + 1],
                    in_=gath[:, b : b + 1],
                    func=mybir.ActivationFunctionType.Ln,
                )

            # --- streaming log + row-sum pass, combine folded in per block ---
            res = small.tile([P, NBLK], mybir.dt.float32)
            for b in range(NBLK):
                chunks = taper_chunks if b == NBLK - 1 else full_chunks
                partial = parts.tile(
                    [P, len(taper_chunks)], mybir.dt.float32, tag="partial"
                )
                col = 0
                for j, ch in enumerate(chunks):
                    t = stream.tile([P, C], mybir.dt.float32, tag="t")
                    nc.sync.dma_start(
                        out=t[:, :ch],
                        in_=pred[b * P : (b + 1) * P, col : col + ch],
                    )
                    nc.scalar.activation(
                        out=t[:, :ch],
                        in_=t[:, :ch],
                        func=mybir.ActivationFunctionType.Ln,
                        accum_out=partial[:, j : j + 1],
                    )
                    col += ch
                bsum = parts.tile([P, 1], mybir.dt.float32, tag="bsum")
                nc.vector.reduce_sum(
                    out=bsum[:],
                    in_=partial[:, : len(chunks)],
                    axis=mybir.AxisListType.X,
                )
                # res[:, b] = (log(gathered) - rowsum) / V
                nc.vector.tensor_scalar(
                    out=res[:, b : b + 1],
                    in0=gath[:, b : b + 1],
                    scalar1=bsum[:],
                    scalar2=1.0 / V,
                    op0=mybir.AluOpType.subtract,
                    op1=mybir.AluOpType.mult,
                )
            # transpose res [128, 8] -> [8, 128] so the output DMA writes
            # 8 contiguous 512B runs instead of 1024 scattered 4B RMW writes
            # (the scattered form cost ~7.5us of write-receipt latency in the
            # kernel-tail drain)
            resT_psum = psum.tile([NBLK, P], mybir.dt.float32)
            nc.tensor.transpose(
                out=resT_psum[:], in_=res[:], identity=identity[:]
            )
            resT = small.tile([NBLK, P], mybir.dt.float32)
            nc.vector.tensor_copy(out=resT[:], in_=resT_psum[:])
            nc.sync.dma_start(
                out=out[:].rearrange("(b p) -> b p", p=P), in_=resT[:]
            )

    nc.compile()
    return nc


def _run(pred, target, trace=False, **kwargs):
    from concourse.bass_utils import run_bass_kernel_spmd

    if "nc" not in _CACHE:
        _CACHE["nc"] = _build_program()
    nc = _CACHE["nc"]

    pred = np.ascontiguousarray(np.asarray(pred, dtype=np.float32))
    tgt = np.asarray(target).astype(np.int64).reshape(-1)
    assert pred.shape == (B, V) and tgt.shape == (B,)

    base = np.arange(R, dtype=np.int64) * V
    in_maps = []
    for c in range(NCORES):
        sl = slice(c * R, (c + 1) * R)
        tidx = (base + tgt[sl]).astype(np.int32)
        in_maps.append({"pred": pred[sl], "tidx": tidx})

    res = run_bass_kernel_spmd(
        nc, in_maps, core_ids=list(range(NCORES)), trace=trace, **kwargs
    )
    out = np.concatenate([np.asarray(r["out"]).reshape(-1) for r in res.results])
    return out, res


def kernel(pred, target):
    return _run(pred, target)[0]
